# revision 38
# baseline (speedup 1.0000x reference)
"""GCN (2-layer, hidden=64, rank-1 weights) on 8 Trainium2 NeuronCores.

Math: both GCNConv layers have rank-1 weight matrices (1->64, 64->1), so each
layer collapses to a scalar SpMV with the symmetric-normalized adjacency
A_hat = D^-1/2 (A+I) D^-1/2:

    s   = A_hat @ x                    (scalar per node)
    z   = f(s)   where f(t) = sum_k W2[k] * relu(W1[k]*t + b1[k])
    out = A_hat @ z + b2

Sharding: nodes are range-sharded by destination across the 8 cores; all
in-edges of a node live on its owner core.  Within a core, nodes are sorted
by in-degree (descending) and rank j maps to SBUF position
(partition j%128, slot j//128), so the occupancy of ELL round r is an
aligned slot prefix of ceil(n_r/128) in every partition.  Rounds are
quantized into a few equal-width groups ("packed ELL"), cutting the routed
table to roughly the true edge count -- less than half the dense-rectangle
bytes.  Within a segment the layout is round-innermost (col = slot*nb + r),
so the whole segment fold is a single DVE tensor_reduce over the innermost
axis, accumulating in f32.  (In-place fp16 tensor_tensor accumulation
chains on narrow regions corrupt on this DVE, so reduction never goes
through them.)

Execution is two SPMD launches (one per GCN layer).  The host routes
per-edge source data to the owning destination core between layers (np.take
-- pure gather, the "halo exchange" of the sharding strategy).  Routed
tables are fp16 so HBM traffic is halved again.  Normalization coefficients
dinv = 1/sqrt(deg+1) are a function of the graph structure only (host
already derives degrees with np.bincount to build the routing tables), so
the host routes dinv[src] per-edge directly; the device performs all
feature arithmetic: the per-edge message products dinv[src]*x[src], the
segment summation (tensor_reduce), the MLP nonlinearity (weight-folded to a
2-segment piecewise-linear map when b1 == 0), the per-node normalization,
the layer-2 message values w = dinv*z, and the bias.

Only the vector (DVE) and sync engines are used: relu is a fused
tensor_scalar (max,mult) on DVE, so no ACT table load.  The dominant
round-group streams in three slot-range DMA chunks so each chunk's
multiply+reduce overlaps the next chunk's transfer; the final chunk carries
the narrow round-groups and the small per-node tables.
"""

import os
import numpy as np

from concourse import bass, mybir
from concourse.bass_utils import run_bass_kernel_spmd

dt = mybir.dt
F16 = np.float16

NCORES = 8
N = 100000
P = 128            # SBUF partitions
CPN = 98           # node slots per partition
NPC = P * CPN      # 12544 nodes per core
SENT = NCORES * NPC  # sentinel table slot (value 0)

LAST_RESULTS = None  # list of BassKernelResults from the most recent run


def _segments(w):
    """Quantize per-round ELL widths into groups, then split the dominant
    first group by slot range for DMA/compute pipelining.

    Returns (segs, dma_groups).  Each seg dict: r0 (first round), nb
    (rounds), s0 (first slot), ns (slots), off (column offset), cols
    (ns*nb padded to even).  dma_groups lists segment indices per DMA
    chunk."""
    K = len(w)
    groups = []
    r = 0
    while r < K:
        if r == 0:
            W = CPN  # round 0 must cover every node slot
        else:
            W = min(CPN, w[r] + (w[r] & 1))
        r2 = r + 1
        thresh = 0.45 if r == 0 else 0.25
        while r2 < K and w[r2] >= thresh * W:
            r2 += 1
        groups.append((r, r2 - r, W))
        r = r2
    while len(groups) > 5:  # at most 4 narrow groups (one scratch tile each)
        (ra, na, Wa), (rb, nb_, Wb) = groups[-2], groups[-1]
        groups[-2:] = [(ra, na + nb_, max(Wa, Wb))]

    segs = []
    g0_r, g0_nb, g0_W = groups[0]
    splits = [0, 32, 64, g0_W] if g0_W > 64 else [0, g0_W]
    for i in range(len(splits) - 1):
        segs.append({"r0": g0_r, "nb": g0_nb,
                     "s0": splits[i], "ns": splits[i + 1] - splits[i]})
    npipe = len(splits) - 1
    for (r0, nb, W) in groups[1:]:
        segs.append({"r0": r0, "nb": nb, "s0": 0, "ns": W})

    off = 0
    for s in segs:
        n = s["ns"] * s["nb"]
        s["off"] = off
        s["cols"] = n + n % 2
        off += s["cols"]

    dma_groups = [[i] for i in range(npipe)]
    tail = list(range(npipe, len(segs)))
    if tail:
        dma_groups.append(tail)
    return segs, dma_groups


def _preprocess(x, edge_index):
    """Host routing/layout: shard by destination, degree-sort nodes, build
    packed-ELL source-index arrays (interleaved rank -> (p, s) mapping,
    round-innermost within each segment)."""
    x = np.asarray(x, dtype=np.float32).reshape(-1)
    ei = np.asarray(edge_index)
    src_g = ei[0].astype(np.int64)
    dst_g = ei[1].astype(np.int64)

    cnt_g = np.bincount(dst_g, minlength=N).astype(np.int64)  # in-degree

    order_c, rank_c, deg_sorted_c = [], [], []
    pp = np.empty(N, dtype=np.int64)  # global node -> permuted table position
    for c in range(NCORES):
        lo, hi = c * NPC, min((c + 1) * NPC, N)
        nreal = hi - lo
        deg_local = np.zeros(NPC, dtype=np.int64)
        deg_local[:nreal] = cnt_g[lo:hi]
        order = np.argsort(-deg_local, kind="stable")
        rank = np.empty(NPC, dtype=np.int64)
        rank[order] = np.arange(NPC)
        order_c.append(order)
        rank_c.append(rank)
        deg_sorted_c.append(deg_local[order])
        pp[lo:hi] = c * NPC + rank[:nreal]

    K = int(max(int(d[0]) for d in deg_sorted_c))  # global max in-degree
    # per-round occupancy width (max over cores), in slots per partition
    w = []
    for r in range(K):
        n_r = max(int(np.searchsorted(-d, -r - 0.5)) for d in deg_sorted_c)
        w.append(min(CPN, (n_r + P - 1) // P))
    segs, dma_groups = _segments(w)

    owner = dst_g // NPC
    idx_c, xs_c, dinv_c = [], [], []
    for c in range(NCORES):
        lo = c * NPC
        m = owner == c
        s_e = pp[src_g[m]]
        d_e = dst_g[m] - lo
        rj = rank_c[c][d_e]
        o = np.argsort(rj, kind="stable")
        rj_s = rj[o]
        s_s = s_e[o]
        occ = np.arange(len(rj_s)) - np.searchsorted(rj_s, rj_s)
        idx_mat = np.full((NPC, K), SENT, dtype=np.int64)
        idx_mat[rj_s, occ] = s_s
        # node rank j -> [p=j%128, col off_seg + (slot-s0)*nb + r']
        i3 = idx_mat.reshape(CPN, P, K)  # [slot, p, round]
        parts = []
        for sg in segs:
            blk = i3[sg["s0"]:sg["s0"] + sg["ns"], :,
                     sg["r0"]:sg["r0"] + sg["nb"]]  # [ns, p, nb]
            pm = blk.transpose(1, 0, 2).reshape(P, sg["ns"] * sg["nb"])
            pad = sg["cols"] - sg["ns"] * sg["nb"]
            if pad:
                pm = np.concatenate(
                    [pm, np.full((P, pad), SENT, dtype=np.int64)], axis=1)
            parts.append(pm)
        idx_c.append(np.ascontiguousarray(np.concatenate(parts, axis=1)))

        nreal = min(NPC, N - lo)
        xv = np.zeros(NPC, dtype=np.float32)
        xv[:nreal] = x[lo:lo + nreal]
        xs_c.append(np.ascontiguousarray(
            xv[order_c[c]].astype(np.float32).reshape(CPN, P).T))
        dinv_c.append(np.ascontiguousarray(
            (1.0 / np.sqrt(deg_sorted_c[c] + 1.0)).astype(np.float32)
            .reshape(CPN, P).T))
    return idx_c, xs_c, dinv_c, rank_c, segs, dma_groups


def _emit_reduce(vector, Y, F, G, sg):
    """One-instruction segment fold: view the segment as [p, ns, nb]
    (round-innermost, contiguous) and reduce the innermost axis into f32.
    Wide (s0-split) segments write their slot range of F directly; each
    narrow segment writes the prefix of its own pre-zeroed scratch tile G
    (summed full-width into the epilogue chain later -- in-place
    accumulation chains on narrow regions corrupt on this DVE)."""
    n = sg["ns"] * sg["nb"]
    v = Y[:, sg["off"]:sg["off"] + n].rearrange(
        "p (s r) -> p s r", s=sg["ns"], r=sg["nb"])
    if sg["r0"] == 0:
        vector.tensor_reduce(
            out=F[:, sg["s0"]:sg["s0"] + sg["ns"]], in_=v,
            axis=mybir.AxisListType.X, op=mybir.AluOpType.add)
    else:
        vector.tensor_reduce(out=G[:, 0:sg["ns"]], in_=v,
                             axis=mybir.AxisListType.X, op=mybir.AluOpType.add)


def _build_layer1(segs, dma_groups, A, B, terms):
    """Layer 1: routed per-edge tables [x[src] | dinv[src]] (fp16, packed
    ELL, in DMA chunks), per-node [x_own | dinv | dinv^2] (fp16, with the
    last chunk).  Output: w_own = dinv * f(s)."""
    nc = bass.Bass(num_devices=NCORES)
    nch = len(dma_groups)
    ccols = [sum(segs[i]["cols"] for i in g) for g in dma_groups]
    esz = [2 * c + (3 * CPN if i == nch - 1 else 0)
           for i, c in enumerate(ccols)]
    L = sum(s["cols"] for s in segs)

    en_in = [nc.declare_dram_parameter(f"en{i}", [P, esz[i]], dt.float16,
                                       isOutput=False)
             for i in range(nch)]
    out_ext = nc.declare_dram_parameter("out", [P, CPN], dt.float32, isOutput=True)

    with (
        nc.sbuf_tensor("E0", [P, esz[0]], dt.float16) as E0,
        nc.sbuf_tensor("E1", [P, esz[min(1, nch - 1)]], dt.float16) as E1,
        nc.sbuf_tensor("E2", [P, esz[min(2, nch - 1)]], dt.float16) as E2,
        nc.sbuf_tensor("E3", [P, esz[min(3, nch - 1)]], dt.float16) as E3,
        nc.sbuf_tensor("Y", [P, L], dt.float16) as Y,
        nc.sbuf_tensor("F", [P, CPN], dt.float32) as F,
        nc.sbuf_tensor("G0", [P, CPN], dt.float32) as G0,
        nc.sbuf_tensor("G1", [P, CPN], dt.float32) as G1,
        nc.sbuf_tensor("G2", [P, CPN], dt.float32) as G2,
        nc.sbuf_tensor("G3", [P, CPN], dt.float32) as G3,
        nc.sbuf_tensor("tb", [P, CPN], dt.float32) as tb,
        nc.sbuf_tensor("tr", [P, CPN], dt.float32) as tr,
        nc.sbuf_tensor("to", [P, CPN], dt.float32) as to,
        nc.sbuf_tensor("wout", [P, CPN], dt.float32) as wout,
        nc.semaphore("s0") as s0,
        nc.semaphore("s1") as s1,
        nc.semaphore("s2") as s2,
        nc.semaphore("s3") as s3,
        nc.semaphore("sp") as sp,
        nc.semaphore("sv") as sv,
        nc.Block() as block,
    ):
        E = [E0, E1, E2, E3][:nch]
        S = [s0, s1, s2, s3][:nch]
        EL = E[nch - 1]
        pbase = 2 * ccols[nch - 1]
        narrow = [sg for sg in segs if sg["r0"] != 0]
        G = [G0, G1, G2, G3]
        gmap = {id(sg): G[k] for k, sg in enumerate(narrow)}

        @block.vector
        def _(vector):
            xo = EL[:, pbase:pbase + CPN]
            do = EL[:, pbase + CPN:pbase + 2 * CPN]
            dd = EL[:, pbase + 2 * CPN:pbase + 3 * CPN]
            for k in range(len(narrow)):  # zero pads while DMAs are in flight
                vector.memset(G[k][:, :], 0.0)
            for i, g in enumerate(dma_groups):
                vector.wait_ge(S[i], 16)
                c = ccols[i]
                off0 = segs[g[0]]["off"]
                # per-edge messages y = dinv[src] * x[src], fp16, fresh dst
                vector.tensor_tensor(
                    out=Y[:, off0:off0 + c],
                    in0=E[i][:, 0:c], in1=E[i][:, c:2 * c],
                    op=mybir.AluOpType.mult)
                for si in g:
                    sg = segs[si]
                    _emit_reduce(vector, Y, F, gmap.get(id(sg)), sg)
            # t = fold + dinv * x_own  (s = dinv * t); all adds fresh-dst
            vector.tensor_tensor(out=tb[:, :], in0=do, in1=xo,
                                 op=mybir.AluOpType.mult)
            vector.tensor_tensor(out=to[:, :], in0=tb[:, :], in1=F[:, :],
                                 op=mybir.AluOpType.add)
            cur, alt = to, tb
            for k in range(len(narrow)):
                vector.tensor_tensor(out=alt[:, :], in0=cur[:, :],
                                     in1=G[k][:, :], op=mybir.AluOpType.add)
                cur, alt = alt, cur
            if terms is None:
                # w = dinv*z = dinv^2 * ((A-B)*relu(t) + B*t)
                #   (relu(dinv*t) = dinv*relu(t) since dinv > 0)
                vector.tensor_scalar(tr[:, :], cur[:, :], 0.0, float(A - B),
                                     mybir.AluOpType.max,
                                     mybir.AluOpType.mult)
                vector.scalar_tensor_tensor(
                    out=tr[:, :], in0=cur[:, :], scalar=float(B), in1=tr[:, :],
                    op0=mybir.AluOpType.mult, op1=mybir.AluOpType.add)
                vector.tensor_tensor(
                    out=wout[:, :], in0=dd, in1=tr[:, :],
                    op=mybir.AluOpType.mult).then_inc(sv, 1)
            else:
                # general path: s = dinv*t, z = sum_k W2k*relu(W1k*s+b1k)
                vector.tensor_tensor(out=alt[:, :], in0=do, in1=cur[:, :],
                                     op=mybir.AluOpType.mult)
                sK = alt
                vector.memset(wout[:, :], 0.0)
                for (w1k, b1k, w2k) in terms:
                    vector.tensor_scalar(
                        tr[:, :], sK[:, :], float(w1k), float(b1k),
                        mybir.AluOpType.mult, mybir.AluOpType.add)
                    vector.tensor_scalar_max(tr[:, :], tr[:, :], 0.0)
                    vector.scalar_tensor_tensor(
                        out=wout[:, :], in0=tr[:, :], scalar=float(w2k),
                        in1=wout[:, :],
                        op0=mybir.AluOpType.mult, op1=mybir.AluOpType.add)
                vector.tensor_tensor(
                    out=wout[:, :], in0=do, in1=wout[:, :],
                    op=mybir.AluOpType.mult).then_inc(sv, 1)

        @block.sync
        def _(sync):
            for i in range(nch):
                sync.dma_start(out=E[i][:, :], in_=en_in[i][:, :]).then_inc(S[i], 16)
            sync.wait_ge(sv, 1)
            sync.dma_start(out=out_ext[:, :], in_=wout[:, :]).then_inc(sp, 16)

    return nc


def _build_layer2(segs, dma_groups, b2val):
    """Layer 2: routed per-edge table w[src] (fp16, packed ELL, DMA'd
    straight into the fold buffer), per-node [w_own | dinv] (fp16, with the
    last chunk).  out = dinv*(sum w_ell + w_own) + b2."""
    nc = bass.Bass(num_devices=NCORES)
    nch = len(dma_groups)
    ccols = [sum(segs[i]["cols"] for i in g) for g in dma_groups]
    wsz = [c + (2 * CPN if i == nch - 1 else 0) for i, c in enumerate(ccols)]
    L = sum(s["cols"] for s in segs)

    we_in = [nc.declare_dram_parameter(f"we{i}", [P, wsz[i]], dt.float16,
                                       isOutput=False)
             for i in range(nch)]
    out_ext = nc.declare_dram_parameter("out", [P, CPN], dt.float32, isOutput=True)

    with (
        nc.sbuf_tensor("Y", [P, L + 2 * CPN], dt.float16) as Y,
        nc.sbuf_tensor("F", [P, CPN], dt.float32) as F,
        nc.sbuf_tensor("G0", [P, CPN], dt.float32) as G0,
        nc.sbuf_tensor("G1", [P, CPN], dt.float32) as G1,
        nc.sbuf_tensor("G2", [P, CPN], dt.float32) as G2,
        nc.sbuf_tensor("G3", [P, CPN], dt.float32) as G3,
        nc.sbuf_tensor("tb", [P, CPN], dt.float32) as tb,
        nc.sbuf_tensor("to", [P, CPN], dt.float32) as to,
        nc.sbuf_tensor("wout", [P, CPN], dt.float32) as wout,
        nc.semaphore("s0") as s0,
        nc.semaphore("s1") as s1,
        nc.semaphore("s2") as s2,
        nc.semaphore("s3") as s3,
        nc.semaphore("sp") as sp,
        nc.semaphore("sv") as sv,
        nc.Block() as block,
    ):
        S = [s0, s1, s2, s3][:nch]
        narrow = [sg for sg in segs if sg["r0"] != 0]
        G = [G0, G1, G2, G3]
        gmap = {id(sg): G[k] for k, sg in enumerate(narrow)}

        @block.vector
        def _(vector):
            wo = Y[:, L:L + CPN]
            do = Y[:, L + CPN:L + 2 * CPN]
            for k in range(len(narrow)):
                vector.memset(G[k][:, :], 0.0)
            for i, g in enumerate(dma_groups):
                vector.wait_ge(S[i], 16)
                for si in g:
                    sg = segs[si]
                    _emit_reduce(vector, Y, F, gmap.get(id(sg)), sg)
            vector.tensor_tensor(out=tb[:, :], in0=F[:, :], in1=wo,
                                 op=mybir.AluOpType.add)
            cur, alt = tb, to
            for k in range(len(narrow)):
                vector.tensor_tensor(out=alt[:, :], in0=cur[:, :],
                                     in1=G[k][:, :], op=mybir.AluOpType.add)
                cur, alt = alt, cur
            vector.tensor_tensor(out=alt[:, :], in0=do, in1=cur[:, :],
                                 op=mybir.AluOpType.mult)
            vector.tensor_scalar_add(wout[:, :], alt[:, :],
                                     float(b2val)).then_inc(sv, 1)

        @block.sync
        def _(sync):
            base = 0
            for i in range(nch):
                sync.dma_start(
                    out=Y[:, base:base + wsz[i]],
                    in_=we_in[i][:, :]).then_inc(S[i], 16)
                base += wsz[i]
            sync.wait_ge(sv, 1)
            sync.dma_start(out=out_ext[:, :], in_=wout[:, :]).then_inc(sp, 16)

    return nc


def kernel(x, edge_index, W1, b1, W2, b2):
    global LAST_RESULTS
    idx_c, xs_c, dinv_c, rank_c, segs, dma_groups = _preprocess(x, edge_index)

    w1 = np.asarray(W1, dtype=np.float64).reshape(-1)
    w2 = np.asarray(W2, dtype=np.float64).reshape(-1)
    b1v = np.asarray(b1, dtype=np.float64).reshape(-1)
    b2v = float(np.asarray(b2, dtype=np.float64).reshape(-1)[0])
    if np.all(b1v == 0.0):
        A = float(np.sum(w2 * w1 * (w1 > 0)))
        B = float(np.sum(w2 * w1 * (w1 < 0)))
        terms = None
    else:
        A = B = 0.0
        terms = [(float(w1[k]), float(b1v[k]), float(w2[k]))
                 for k in range(len(w1))]

    # routed tables in permuted (per-core degree-sorted) rank order + sentinel
    x_tab = np.zeros(SENT + 1, dtype=np.float32)
    d_tab = np.zeros(SENT + 1, dtype=np.float32)
    for c in range(NCORES):
        x_tab[c * NPC:(c + 1) * NPC] = xs_c[c].T.reshape(-1)
        d_tab[c * NPC:(c + 1) * NPC] = dinv_c[c].T.reshape(-1)
    x_tab16 = x_tab.astype(F16)
    d_tab16 = d_tab.astype(F16)

    # column ranges of each DMA chunk within the packed idx array
    nch = len(dma_groups)
    cuts = []
    for g in dma_groups:
        lo = segs[g[0]]["off"]
        hi = segs[g[-1]]["off"] + segs[g[-1]]["cols"]
        cuts.append((lo, hi))

    trace = bool(os.environ.get("BASS_TRACE"))

    # ---- layer 1 ----
    nc1 = _build_layer1(segs, dma_groups, A, B, terms)
    maps1 = []
    for c in range(NCORES):
        m = {}
        for i, (lo, hi) in enumerate(cuts):
            cols = idx_c[c][:, lo:hi]
            parts = [x_tab16[cols], d_tab16[cols]]
            if i == nch - 1:
                parts += [xs_c[c].astype(F16), dinv_c[c].astype(F16),
                          (dinv_c[c] * dinv_c[c]).astype(F16)]
            m[f"en{i}"] = np.ascontiguousarray(np.concatenate(parts, axis=1))
        maps1.append(m)
    res1 = run_bass_kernel_spmd(nc1, maps1, list(range(NCORES)), trace=trace)

    # host routes layer-1 message values to edge slots (halo exchange)
    w_tab = np.zeros(SENT + 1, dtype=np.float32)
    w_own_c = []
    for c in range(NCORES):
        wv = np.asarray(res1.results[c]["out"])  # [P, CPN], node at [p, s]
        w_own_c.append(np.ascontiguousarray(wv.astype(np.float32)))
        w_tab[c * NPC:(c + 1) * NPC] = wv.T.reshape(-1)
    w_tab16 = w_tab.astype(F16)

    # ---- layer 2 ----
    nc2 = _build_layer2(segs, dma_groups, b2v)
    maps2 = []
    for c in range(NCORES):
        m = {}
        for i, (lo, hi) in enumerate(cuts):
            parts = [w_tab16[idx_c[c][:, lo:hi]]]
            if i == nch - 1:
                parts += [w_own_c[c].astype(F16), dinv_c[c].astype(F16)]
            m[f"we{i}"] = np.ascontiguousarray(np.concatenate(parts, axis=1))
        maps2.append(m)
    res2 = run_bass_kernel_spmd(nc2, maps2, list(range(NCORES)), trace=trace)

    LAST_RESULTS = [res1, res2]

    out = np.empty((N, 1), dtype=np.float32)
    for c in range(NCORES):
        lo, hi = c * NPC, min((c + 1) * NPC, N)
        o_rank = np.asarray(res2.results[c]["out"]).T.reshape(NPC)
        out[lo:hi, 0] = o_rank[rank_c[c][:hi - lo]]
    return out


# revision 39
# speedup vs baseline: 1.0105x; 1.0105x over previous
"""GCN (2-layer, hidden=64, rank-1 weights) on 8 Trainium2 NeuronCores.

Math: both GCNConv layers have rank-1 weight matrices (1->64, 64->1), so each
layer collapses to a scalar SpMV with the symmetric-normalized adjacency
A_hat = D^-1/2 (A+I) D^-1/2:

    s   = A_hat @ x                    (scalar per node)
    z   = f(s)   where f(t) = sum_k W2[k] * relu(W1[k]*t + b1[k])
    out = A_hat @ z + b2

Sharding: nodes are range-sharded by destination across the 8 cores; all
in-edges of a node live on its owner core.  Within a core, nodes are sorted
by in-degree (descending) and rank j maps to SBUF position
(partition j%128, slot j//128), so the occupancy of ELL round r is an
aligned slot prefix of ceil(n_r/128) in every partition.  Rounds are
quantized into a few equal-width groups ("packed ELL"), cutting the routed
table to roughly the true edge count -- less than half the dense-rectangle
bytes.  Within a segment the layout is round-innermost (col = slot*nb + r),
so the whole segment fold is a single DVE tensor_reduce over the innermost
axis, accumulating in f32.  (In-place fp16 tensor_tensor accumulation
chains on narrow regions corrupt on this DVE, so reduction never goes
through them.)

Execution is two SPMD launches (one per GCN layer).  The host routes
per-edge source data to the owning destination core between layers (np.take
-- pure gather, the "halo exchange" of the sharding strategy).  Routed
tables are fp16 so HBM traffic is halved again.  Normalization coefficients
dinv = 1/sqrt(deg+1) are a function of the graph structure only (host
already derives degrees with np.bincount to build the routing tables), so
the host routes dinv[src] per-edge directly; the device performs all
feature arithmetic: the per-edge message products dinv[src]*x[src], the
segment summation (tensor_reduce), the MLP nonlinearity (weight-folded to a
2-segment piecewise-linear map when b1 == 0), the per-node normalization,
the layer-2 message values w = dinv*z, and the bias.

Only the vector (DVE) and sync engines are used: relu is a fused
tensor_scalar (max,mult) on DVE, so no ACT table load.  The dominant
round-group streams in three slot-range DMA chunks so each chunk's
multiply+reduce overlaps the next chunk's transfer; the final chunk carries
the narrow round-groups and the small per-node tables.
"""

import os
import numpy as np

from concourse import bass, mybir
from concourse.bass_utils import run_bass_kernel_spmd

dt = mybir.dt
F16 = np.float16

NCORES = 8
N = 100000
P = 128            # SBUF partitions
CPN = 98           # node slots per partition
NPC = P * CPN      # 12544 nodes per core
SENT = NCORES * NPC  # sentinel table slot (value 0)

LAST_RESULTS = None  # list of BassKernelResults from the most recent run


def _segments(w):
    """Quantize per-round ELL widths into groups, then split the dominant
    first group by slot range for DMA/compute pipelining.

    Returns (segs, dma_groups).  Each seg dict: r0 (first round), nb
    (rounds), s0 (first slot), ns (slots), off (column offset), cols
    (ns*nb padded to even).  dma_groups lists segment indices per DMA
    chunk."""
    K = len(w)
    groups = []
    r = 0
    while r < K:
        if r == 0:
            W = CPN  # round 0 must cover every node slot
        else:
            W = min(CPN, w[r] + (w[r] & 1))
        r2 = r + 1
        thresh = 0.45 if r == 0 else 0.15
        while r2 < K and w[r2] >= thresh * W:
            r2 += 1
        groups.append((r, r2 - r, W))
        r = r2
    while len(groups) > 5:  # at most 4 narrow groups (one scratch tile each)
        (ra, na, Wa), (rb, nb_, Wb) = groups[-2], groups[-1]
        groups[-2:] = [(ra, na + nb_, max(Wa, Wb))]

    segs = []
    g0_r, g0_nb, g0_W = groups[0]
    splits = [0, 32, 64, g0_W] if g0_W > 64 else [0, g0_W]
    for i in range(len(splits) - 1):
        segs.append({"r0": g0_r, "nb": g0_nb,
                     "s0": splits[i], "ns": splits[i + 1] - splits[i]})
    npipe = len(splits) - 1
    for (r0, nb, W) in groups[1:]:
        segs.append({"r0": r0, "nb": nb, "s0": 0, "ns": W})

    off = 0
    for s in segs:
        n = s["ns"] * s["nb"]
        s["off"] = off
        s["cols"] = n + n % 2
        off += s["cols"]

    dma_groups = [[i] for i in range(npipe)]
    tail = list(range(npipe, len(segs)))
    if tail:
        dma_groups.append(tail)
    return segs, dma_groups


def _preprocess(x, edge_index):
    """Host routing/layout: shard by destination, degree-sort nodes, build
    packed-ELL source-index arrays (interleaved rank -> (p, s) mapping,
    round-innermost within each segment)."""
    x = np.asarray(x, dtype=np.float32).reshape(-1)
    ei = np.asarray(edge_index)
    src_g = ei[0].astype(np.int64)
    dst_g = ei[1].astype(np.int64)

    cnt_g = np.bincount(dst_g, minlength=N).astype(np.int64)  # in-degree

    order_c, rank_c, deg_sorted_c = [], [], []
    pp = np.empty(N, dtype=np.int64)  # global node -> permuted table position
    for c in range(NCORES):
        lo, hi = c * NPC, min((c + 1) * NPC, N)
        nreal = hi - lo
        deg_local = np.zeros(NPC, dtype=np.int64)
        deg_local[:nreal] = cnt_g[lo:hi]
        order = np.argsort(-deg_local, kind="stable")
        rank = np.empty(NPC, dtype=np.int64)
        rank[order] = np.arange(NPC)
        order_c.append(order)
        rank_c.append(rank)
        deg_sorted_c.append(deg_local[order])
        pp[lo:hi] = c * NPC + rank[:nreal]

    K = int(max(int(d[0]) for d in deg_sorted_c))  # global max in-degree
    # per-round occupancy width (max over cores), in slots per partition
    w = []
    for r in range(K):
        n_r = max(int(np.searchsorted(-d, -r - 0.5)) for d in deg_sorted_c)
        w.append(min(CPN, (n_r + P - 1) // P))
    segs, dma_groups = _segments(w)

    owner = dst_g // NPC
    idx_c, xs_c, dinv_c = [], [], []
    for c in range(NCORES):
        lo = c * NPC
        m = owner == c
        s_e = pp[src_g[m]]
        d_e = dst_g[m] - lo
        rj = rank_c[c][d_e]
        o = np.argsort(rj, kind="stable")
        rj_s = rj[o]
        s_s = s_e[o]
        occ = np.arange(len(rj_s)) - np.searchsorted(rj_s, rj_s)
        idx_mat = np.full((NPC, K), SENT, dtype=np.int64)
        idx_mat[rj_s, occ] = s_s
        # node rank j -> [p=j%128, col off_seg + (slot-s0)*nb + r']
        i3 = idx_mat.reshape(CPN, P, K)  # [slot, p, round]
        parts = []
        for sg in segs:
            blk = i3[sg["s0"]:sg["s0"] + sg["ns"], :,
                     sg["r0"]:sg["r0"] + sg["nb"]]  # [ns, p, nb]
            pm = blk.transpose(1, 0, 2).reshape(P, sg["ns"] * sg["nb"])
            pad = sg["cols"] - sg["ns"] * sg["nb"]
            if pad:
                pm = np.concatenate(
                    [pm, np.full((P, pad), SENT, dtype=np.int64)], axis=1)
            parts.append(pm)
        idx_c.append(np.ascontiguousarray(np.concatenate(parts, axis=1)))

        nreal = min(NPC, N - lo)
        xv = np.zeros(NPC, dtype=np.float32)
        xv[:nreal] = x[lo:lo + nreal]
        xs_c.append(np.ascontiguousarray(
            xv[order_c[c]].astype(np.float32).reshape(CPN, P).T))
        dinv_c.append(np.ascontiguousarray(
            (1.0 / np.sqrt(deg_sorted_c[c] + 1.0)).astype(np.float32)
            .reshape(CPN, P).T))
    return idx_c, xs_c, dinv_c, rank_c, segs, dma_groups


def _emit_reduce(vector, Y, F, G, sg):
    """One-instruction segment fold: view the segment as [p, ns, nb]
    (round-innermost, contiguous) and reduce the innermost axis into f32.
    Wide (s0-split) segments write their slot range of F directly; each
    narrow segment writes the prefix of its own pre-zeroed scratch tile G
    (summed full-width into the epilogue chain later -- in-place
    accumulation chains on narrow regions corrupt on this DVE)."""
    n = sg["ns"] * sg["nb"]
    v = Y[:, sg["off"]:sg["off"] + n].rearrange(
        "p (s r) -> p s r", s=sg["ns"], r=sg["nb"])
    if sg["r0"] == 0:
        vector.tensor_reduce(
            out=F[:, sg["s0"]:sg["s0"] + sg["ns"]], in_=v,
            axis=mybir.AxisListType.X, op=mybir.AluOpType.add)
    else:
        vector.tensor_reduce(out=G[:, 0:sg["ns"]], in_=v,
                             axis=mybir.AxisListType.X, op=mybir.AluOpType.add)


def _build_layer1(segs, dma_groups, A, B, terms):
    """Layer 1: routed per-edge tables [x[src] | dinv[src]] (fp16, packed
    ELL, in DMA chunks), per-node [x_own | dinv | dinv^2] (fp16, with the
    last chunk).  Output: w_own = dinv * f(s)."""
    nc = bass.Bass(num_devices=NCORES)
    nch = len(dma_groups)
    ccols = [sum(segs[i]["cols"] for i in g) for g in dma_groups]
    esz = [2 * c + (3 * CPN if i == nch - 1 else 0)
           for i, c in enumerate(ccols)]
    L = sum(s["cols"] for s in segs)

    en_in = [nc.declare_dram_parameter(f"en{i}", [P, esz[i]], dt.float16,
                                       isOutput=False)
             for i in range(nch)]
    out_ext = nc.declare_dram_parameter("out", [P, CPN], dt.float32, isOutput=True)

    with (
        nc.sbuf_tensor("E0", [P, esz[0]], dt.float16) as E0,
        nc.sbuf_tensor("E1", [P, esz[min(1, nch - 1)]], dt.float16) as E1,
        nc.sbuf_tensor("E2", [P, esz[min(2, nch - 1)]], dt.float16) as E2,
        nc.sbuf_tensor("E3", [P, esz[min(3, nch - 1)]], dt.float16) as E3,
        nc.sbuf_tensor("Y", [P, L], dt.float16) as Y,
        nc.sbuf_tensor("F", [P, CPN], dt.float32) as F,
        nc.sbuf_tensor("G0", [P, CPN], dt.float32) as G0,
        nc.sbuf_tensor("G1", [P, CPN], dt.float32) as G1,
        nc.sbuf_tensor("G2", [P, CPN], dt.float32) as G2,
        nc.sbuf_tensor("G3", [P, CPN], dt.float32) as G3,
        nc.sbuf_tensor("tb", [P, CPN], dt.float32) as tb,
        nc.sbuf_tensor("tr", [P, CPN], dt.float32) as tr,
        nc.sbuf_tensor("to", [P, CPN], dt.float32) as to,
        nc.sbuf_tensor("wout", [P, CPN], dt.float32) as wout,
        nc.semaphore("s0") as s0,
        nc.semaphore("s1") as s1,
        nc.semaphore("s2") as s2,
        nc.semaphore("s3") as s3,
        nc.semaphore("sp") as sp,
        nc.semaphore("sv") as sv,
        nc.Block() as block,
    ):
        E = [E0, E1, E2, E3][:nch]
        S = [s0, s1, s2, s3][:nch]
        EL = E[nch - 1]
        pbase = 2 * ccols[nch - 1]
        narrow = [sg for sg in segs if sg["r0"] != 0]
        G = [G0, G1, G2, G3]
        gmap = {id(sg): G[k] for k, sg in enumerate(narrow)}

        @block.vector
        def _(vector):
            xo = EL[:, pbase:pbase + CPN]
            do = EL[:, pbase + CPN:pbase + 2 * CPN]
            dd = EL[:, pbase + 2 * CPN:pbase + 3 * CPN]
            for k in range(len(narrow)):  # zero pads while DMAs are in flight
                vector.memset(G[k][:, :], 0.0)
            for i, g in enumerate(dma_groups):
                vector.wait_ge(S[i], 16)
                c = ccols[i]
                off0 = segs[g[0]]["off"]
                # per-edge messages y = dinv[src] * x[src], fp16, fresh dst
                vector.tensor_tensor(
                    out=Y[:, off0:off0 + c],
                    in0=E[i][:, 0:c], in1=E[i][:, c:2 * c],
                    op=mybir.AluOpType.mult)
                for si in g:
                    sg = segs[si]
                    _emit_reduce(vector, Y, F, gmap.get(id(sg)), sg)
            # t = fold + dinv * x_own  (s = dinv * t); all adds fresh-dst
            vector.tensor_tensor(out=tb[:, :], in0=do, in1=xo,
                                 op=mybir.AluOpType.mult)
            vector.tensor_tensor(out=to[:, :], in0=tb[:, :], in1=F[:, :],
                                 op=mybir.AluOpType.add)
            cur, alt = to, tb
            for k in range(len(narrow)):
                vector.tensor_tensor(out=alt[:, :], in0=cur[:, :],
                                     in1=G[k][:, :], op=mybir.AluOpType.add)
                cur, alt = alt, cur
            if terms is None:
                # w = dinv*z = dinv^2 * ((A-B)*relu(t) + B*t)
                #   (relu(dinv*t) = dinv*relu(t) since dinv > 0)
                vector.tensor_scalar(tr[:, :], cur[:, :], 0.0, float(A - B),
                                     mybir.AluOpType.max,
                                     mybir.AluOpType.mult)
                vector.scalar_tensor_tensor(
                    out=tr[:, :], in0=cur[:, :], scalar=float(B), in1=tr[:, :],
                    op0=mybir.AluOpType.mult, op1=mybir.AluOpType.add)
                vector.tensor_tensor(
                    out=wout[:, :], in0=dd, in1=tr[:, :],
                    op=mybir.AluOpType.mult).then_inc(sv, 1)
            else:
                # general path: s = dinv*t, z = sum_k W2k*relu(W1k*s+b1k)
                vector.tensor_tensor(out=alt[:, :], in0=do, in1=cur[:, :],
                                     op=mybir.AluOpType.mult)
                sK = alt
                vector.memset(wout[:, :], 0.0)
                for (w1k, b1k, w2k) in terms:
                    vector.tensor_scalar(
                        tr[:, :], sK[:, :], float(w1k), float(b1k),
                        mybir.AluOpType.mult, mybir.AluOpType.add)
                    vector.tensor_scalar_max(tr[:, :], tr[:, :], 0.0)
                    vector.scalar_tensor_tensor(
                        out=wout[:, :], in0=tr[:, :], scalar=float(w2k),
                        in1=wout[:, :],
                        op0=mybir.AluOpType.mult, op1=mybir.AluOpType.add)
                vector.tensor_tensor(
                    out=wout[:, :], in0=do, in1=wout[:, :],
                    op=mybir.AluOpType.mult).then_inc(sv, 1)

        @block.sync
        def _(sync):
            for i in range(nch):
                sync.dma_start(out=E[i][:, :], in_=en_in[i][:, :]).then_inc(S[i], 16)
            sync.wait_ge(sv, 1)
            sync.dma_start(out=out_ext[:, :], in_=wout[:, :]).then_inc(sp, 16)

    return nc


def _build_layer2(segs, dma_groups, b2val):
    """Layer 2: routed per-edge table w[src] (fp16, packed ELL, DMA'd
    straight into the fold buffer), per-node [w_own | dinv] (fp16, with the
    last chunk).  out = dinv*(sum w_ell + w_own) + b2."""
    nc = bass.Bass(num_devices=NCORES)
    nch = len(dma_groups)
    ccols = [sum(segs[i]["cols"] for i in g) for g in dma_groups]
    wsz = [c + (2 * CPN if i == nch - 1 else 0) for i, c in enumerate(ccols)]
    L = sum(s["cols"] for s in segs)

    we_in = [nc.declare_dram_parameter(f"we{i}", [P, wsz[i]], dt.float16,
                                       isOutput=False)
             for i in range(nch)]
    out_ext = nc.declare_dram_parameter("out", [P, CPN], dt.float32, isOutput=True)

    with (
        nc.sbuf_tensor("Y", [P, L + 2 * CPN], dt.float16) as Y,
        nc.sbuf_tensor("F", [P, CPN], dt.float32) as F,
        nc.sbuf_tensor("G0", [P, CPN], dt.float32) as G0,
        nc.sbuf_tensor("G1", [P, CPN], dt.float32) as G1,
        nc.sbuf_tensor("G2", [P, CPN], dt.float32) as G2,
        nc.sbuf_tensor("G3", [P, CPN], dt.float32) as G3,
        nc.sbuf_tensor("tb", [P, CPN], dt.float32) as tb,
        nc.sbuf_tensor("to", [P, CPN], dt.float32) as to,
        nc.sbuf_tensor("wout", [P, CPN], dt.float32) as wout,
        nc.semaphore("s0") as s0,
        nc.semaphore("s1") as s1,
        nc.semaphore("s2") as s2,
        nc.semaphore("s3") as s3,
        nc.semaphore("sp") as sp,
        nc.semaphore("sv") as sv,
        nc.Block() as block,
    ):
        S = [s0, s1, s2, s3][:nch]
        narrow = [sg for sg in segs if sg["r0"] != 0]
        G = [G0, G1, G2, G3]
        gmap = {id(sg): G[k] for k, sg in enumerate(narrow)}

        @block.vector
        def _(vector):
            wo = Y[:, L:L + CPN]
            do = Y[:, L + CPN:L + 2 * CPN]
            for k in range(len(narrow)):
                vector.memset(G[k][:, :], 0.0)
            for i, g in enumerate(dma_groups):
                vector.wait_ge(S[i], 16)
                for si in g:
                    sg = segs[si]
                    _emit_reduce(vector, Y, F, gmap.get(id(sg)), sg)
            vector.tensor_tensor(out=tb[:, :], in0=F[:, :], in1=wo,
                                 op=mybir.AluOpType.add)
            cur, alt = tb, to
            for k in range(len(narrow)):
                vector.tensor_tensor(out=alt[:, :], in0=cur[:, :],
                                     in1=G[k][:, :], op=mybir.AluOpType.add)
                cur, alt = alt, cur
            vector.tensor_tensor(out=alt[:, :], in0=do, in1=cur[:, :],
                                 op=mybir.AluOpType.mult)
            vector.tensor_scalar_add(wout[:, :], alt[:, :],
                                     float(b2val)).then_inc(sv, 1)

        @block.sync
        def _(sync):
            base = 0
            for i in range(nch):
                sync.dma_start(
                    out=Y[:, base:base + wsz[i]],
                    in_=we_in[i][:, :]).then_inc(S[i], 16)
                base += wsz[i]
            sync.wait_ge(sv, 1)
            sync.dma_start(out=out_ext[:, :], in_=wout[:, :]).then_inc(sp, 16)

    return nc


def kernel(x, edge_index, W1, b1, W2, b2):
    global LAST_RESULTS
    idx_c, xs_c, dinv_c, rank_c, segs, dma_groups = _preprocess(x, edge_index)

    w1 = np.asarray(W1, dtype=np.float64).reshape(-1)
    w2 = np.asarray(W2, dtype=np.float64).reshape(-1)
    b1v = np.asarray(b1, dtype=np.float64).reshape(-1)
    b2v = float(np.asarray(b2, dtype=np.float64).reshape(-1)[0])
    if np.all(b1v == 0.0):
        A = float(np.sum(w2 * w1 * (w1 > 0)))
        B = float(np.sum(w2 * w1 * (w1 < 0)))
        terms = None
    else:
        A = B = 0.0
        terms = [(float(w1[k]), float(b1v[k]), float(w2[k]))
                 for k in range(len(w1))]

    # routed tables in permuted (per-core degree-sorted) rank order + sentinel
    x_tab = np.zeros(SENT + 1, dtype=np.float32)
    d_tab = np.zeros(SENT + 1, dtype=np.float32)
    for c in range(NCORES):
        x_tab[c * NPC:(c + 1) * NPC] = xs_c[c].T.reshape(-1)
        d_tab[c * NPC:(c + 1) * NPC] = dinv_c[c].T.reshape(-1)
    x_tab16 = x_tab.astype(F16)
    d_tab16 = d_tab.astype(F16)

    # column ranges of each DMA chunk within the packed idx array
    nch = len(dma_groups)
    cuts = []
    for g in dma_groups:
        lo = segs[g[0]]["off"]
        hi = segs[g[-1]]["off"] + segs[g[-1]]["cols"]
        cuts.append((lo, hi))

    trace = bool(os.environ.get("BASS_TRACE"))

    # ---- layer 1 ----
    nc1 = _build_layer1(segs, dma_groups, A, B, terms)
    maps1 = []
    for c in range(NCORES):
        m = {}
        for i, (lo, hi) in enumerate(cuts):
            cols = idx_c[c][:, lo:hi]
            parts = [x_tab16[cols], d_tab16[cols]]
            if i == nch - 1:
                parts += [xs_c[c].astype(F16), dinv_c[c].astype(F16),
                          (dinv_c[c] * dinv_c[c]).astype(F16)]
            m[f"en{i}"] = np.ascontiguousarray(np.concatenate(parts, axis=1))
        maps1.append(m)
    res1 = run_bass_kernel_spmd(nc1, maps1, list(range(NCORES)), trace=trace)

    # host routes layer-1 message values to edge slots (halo exchange)
    w_tab = np.zeros(SENT + 1, dtype=np.float32)
    w_own_c = []
    for c in range(NCORES):
        wv = np.asarray(res1.results[c]["out"])  # [P, CPN], node at [p, s]
        w_own_c.append(np.ascontiguousarray(wv.astype(np.float32)))
        w_tab[c * NPC:(c + 1) * NPC] = wv.T.reshape(-1)
    w_tab16 = w_tab.astype(F16)

    # ---- layer 2 ----
    nc2 = _build_layer2(segs, dma_groups, b2v)
    maps2 = []
    for c in range(NCORES):
        m = {}
        for i, (lo, hi) in enumerate(cuts):
            parts = [w_tab16[idx_c[c][:, lo:hi]]]
            if i == nch - 1:
                parts += [w_own_c[c].astype(F16), dinv_c[c].astype(F16)]
            m[f"we{i}"] = np.ascontiguousarray(np.concatenate(parts, axis=1))
        maps2.append(m)
    res2 = run_bass_kernel_spmd(nc2, maps2, list(range(NCORES)), trace=trace)

    LAST_RESULTS = [res1, res2]

    out = np.empty((N, 1), dtype=np.float32)
    for c in range(NCORES):
        lo, hi = c * NPC, min((c + 1) * NPC, N)
        o_rank = np.asarray(res2.results[c]["out"]).T.reshape(NPC)
        out[lo:hi, 0] = o_rank[rank_c[c][:hi - lo]]
    return out
